# revision 1
# baseline (speedup 1.0000x reference)
"""Trainium2 Bass kernel for nn_EnhancedTFNLayer (RBF field projection +
diffusion + sampling + LN/linear epilogue), data-parallel over batch on 8 cores.

Approach: the RBF kernel family exp(-(p-g)^2/(2 sigma^2)) over the uniform
grid has low numerical rank. We build (on host, float64, from the *parameter*
inputs only) an orthonormal basis Q [R, G] for field functions, plus fitted
operators so the whole pipeline becomes small R-dim matmuls on device:

  phi[n, j] = exp(-(p_n - c_j)^2 / (2 s^2))     (anchor features, K=3 matmul + Exp)
  C_raw = phi^T @ emb          [R, D]
  C     = Wq^T @ C_raw         (orthonormal coords; field(g) ~= Q[:,g]^T C)
  4x:   T = tanh(Q^T (C W_int) + b_int);  C' = SL C + DT * (Q @ T)
  sampled = phi @ (MQ @ C)     (fitted linear-interp evaluation)
  out = LN2(LN1(sampled + emb) @ W_out + b_out + LN1(...))
"""
import sys
import hashlib
import numpy as np

for _p in ("/opt/trn_rl_repo", "/root/.axon_site/_ro/trn_rl_repo"):
    if _p not in sys.path:
        sys.path.insert(0, _p)

import concourse.bass as bass
import concourse.bacc as bacc
import concourse.tile as tile
from concourse import mybir

F32 = mybir.dt.float32
F32R = mybir.dt.float32r
ACTF = mybir.ActivationFunctionType
ALU = mybir.AluOpType

B, N, G, D = 16, 4096, 1024, 256
NUM_STEPS, DT, EPS = 4, 0.01, 1e-5
R = 128
NT = N // 128            # 32 token tiles per batch
NCHUNK = 8               # phi^T chunks of 512 tokens
BL = 2                   # batches per core
NCORES = 8

_CACHE = {}


# --------------------------------------------------------------------------
# host-side operator fitting (float64; parameter inputs only)
# --------------------------------------------------------------------------
def _host_plan(sigma, alpha, grid, W_int, b_int, W_out, b_out,
               ln1_g, ln1_b, ln2_g, ln2_b):
    rng = np.random.default_rng(0)
    c0 = 1.0 - 2.0 * alpha * DT
    c1 = alpha * DT
    pg = np.linspace(0.0, 1.0, 8193)
    K = np.exp(-((pg[:, None] - grid[None, :]) ** 2) / (2 * sigma * sigma))
    # basis enrichment with synthetic tanh fields (params only, no data)
    nsyn = 384
    sub = rng.choice(len(pg), size=256, replace=False)
    Fsyn = K[sub].T @ rng.standard_normal((256, nsyn))
    Fsyn /= np.abs(Fsyn).max(0, keepdims=True) + 1e-30
    fscale = np.sqrt(N * sigma * np.sqrt(np.pi))          # ~field magnitude per unit emb std
    wnorm = np.linalg.norm(W_int, axis=0)
    wcols = rng.choice(len(wnorm), size=nsyn)
    gains = fscale * wnorm[wcols] * rng.uniform(0.5, 2.0, nsyn)
    Tsyn = np.tanh(Fsyn * gains[None, :])
    Msvd = np.concatenate([K, (Tsyn * 0.1).T], axis=0)
    _, _, Vt = np.linalg.svd(Msvd, full_matrices=False)
    Q = Vt[:R]                                            # [R, G] orthonormal rows
    # anchors
    c = np.linspace(-0.08, 1.08, R)
    s = 2.2 * (c[1] - c[0])
    F = np.exp(-((pg[:, None] - c[None, :]) ** 2) / (2 * s * s))
    Qk = K @ Q.T
    Wq, *_ = np.linalg.lstsq(F, Qk, rcond=1e-8)           # [R, R]
    # diffusion operator in Q coords (exact edge-padded 3-tap applied to Q^T)
    Qt = Q.T
    LQt = c0 * Qt.copy()
    LQt[1:-1] += c1 * (Qt[:-2] + Qt[2:])
    LQt[0] += c1 * (Qt[0] + Qt[1])
    LQt[-1] += c1 * (Qt[-2] + Qt[-1])
    SLQ = Q @ LQt                                         # [R, R]
    # sampling (linear interp of Q columns) fitted over anchors
    u = pg * (G - 1)
    i0 = np.clip(np.floor(u), 0, G - 2).astype(int)
    w = u - i0
    lerpQ = Qt[i0] * (1 - w)[:, None] + Qt[i0 + 1] * w[:, None]
    MQ, *_ = np.linalg.lstsq(F, lerpQ, rcond=1e-5)        # [R, R]

    f32 = lambda x: np.ascontiguousarray(x, dtype=np.float32)
    # f32r blob [128, 3584]: q_sb | qt_proj | slt | wq | mqt | wi | wo | ident
    cr = np.concatenate([
        Q,                                                    # q_sb [128,1024]
        (Qt * DT).reshape(8, 128, R).transpose(1, 0, 2).reshape(128, 8 * R),  # qt_proj
        SLQ.T, Wq, MQ.T,                                      # slt, wq, mqt
        W_int.reshape(2, 128, D).transpose(1, 0, 2).reshape(128, 2 * D),      # wi
        W_out.reshape(2, 128, D).transpose(1, 0, 2).reshape(128, 2 * D),      # wo
        np.eye(128),                                          # ident
    ], axis=1)
    # f32 blob [128, 1025]: g1|b1|g2|b2|epsb
    cg = np.concatenate([
        np.broadcast_to(ln1_g, (128, D)), np.broadcast_to(ln1_b, (128, D)),
        np.broadcast_to(ln2_g, (128, D)), np.broadcast_to(ln2_b, (128, D)),
        np.full((128, 1), EPS),
    ], axis=1)
    # row blob [1, 4736]: ones_row|bint|bout|ones_col
    crow = np.concatenate([
        np.ones((1, N)), b_int.reshape(1, D), b_out.reshape(1, D),
        np.ones((1, 128)),
    ], axis=1)
    consts = {
        # phi exponent = p*a1_j + 1*a2_j + p^2*a3 : rhs [3, R] for K=3 matmul
        "anch": f32(np.stack([c / (s * s),
                              -c * c / (2 * s * s),
                              np.full(R, -1.0 / (2 * s * s))])),
        "cr": f32(cr),
        "cg": f32(cg),
        "crow": f32(crow),
    }
    flags = {
        "use_bint": bool(np.any(b_int != 0)),
        "use_bout": bool(np.any(b_out != 0)),
        "ln1_aff": bool(np.any(ln1_g != 1) or np.any(ln1_b != 0)),
        "ln2_aff": bool(np.any(ln2_g != 1) or np.any(ln2_b != 0)),
    }
    return consts, flags


# --------------------------------------------------------------------------
# device module
# --------------------------------------------------------------------------
def _build_module(flags, repeats=1, parts=("s1", "diff", "epi")):
    nc = bacc.Bacc(trn_type="TRN2")
    dt_in = {}
    # inputs
    emb_d = nc.dram_tensor("emb", [BL, N, D], F32R, kind="ExternalInput")
    pos_d = nc.dram_tensor("pos", [BL, N, 1], F32, kind="ExternalInput")
    const_specs = {
        "anch": ([3, R], F32),
        "cr": ([128, 3584], F32R),
        "cg": ([128, 1025], F32),
        "crow": ([1, N + 2 * D + 128], F32),
    }
    cd = {k: nc.dram_tensor(k, sh, dt, kind="ExternalInput")
          for k, (sh, dt) in const_specs.items()}
    out_d = nc.dram_tensor("out", [BL, N, D], F32, kind="ExternalOutput")
    scratch_d = nc.dram_tensor("scratch", [BL, N], F32, kind="Internal")

    with tile.TileContext(nc) as tc:
        with tc.tile_pool(name="consts", bufs=1) as cp, \
             tc.tile_pool(name="emb", bufs=2) as embp, \
             tc.tile_pool(name="phit", bufs=2) as phitp, \
             tc.tile_pool(name="coef", bufs=2) as coefp, \
             tc.tile_pool(name="pre", bufs=2) as prep, \
             tc.tile_pool(name="work", bufs=3) as wp, \
             tc.tile_pool(name="tiny", bufs=8) as tp, \
             tc.tile_pool(name="ppA", bufs=1, space="PSUM") as ppA, \
             tc.tile_pool(name="ppB", bufs=1, space="PSUM") as ppB:

            # ---- load constants (4 DMAs) then carve views ----
            blob = {}
            for k, (sh, dt) in const_specs.items():
                if k == "crow":
                    blob[k] = cp.tile([1, 2 * D + 128], F32, tag=k, name=f"c_{k}")
                    nc.sync.dma_start(blob[k][:], cd[k][:, N:])
                else:
                    blob[k] = cp.tile(sh, dt, tag=k, name=f"c_{k}")
                    nc.sync.dma_start(blob[k][:], cd[k][tuple(slice(None) for _ in sh)])
            _cr, _cg, _crow = blob["cr"], blob["cg"], blob["crow"]
            ct = {
                "anch": blob["anch"],
                "q_sb": _cr[:, 0:1024],
                "qt_proj": _cr[:, 1024:2048].rearrange("p (a b) -> p a b", a=8),
                "slt": _cr[:, 2048:2176], "wq": _cr[:, 2176:2304],
                "mqt": _cr[:, 2304:2432],
                "wi": _cr[:, 2432:2944].rearrange("p (a b) -> p a b", a=2),
                "wo": _cr[:, 2944:3456].rearrange("p (a b) -> p a b", a=2),
                "ident": _cr[:, 3456:3584],
                "g1": _cg[:, 0:256], "b1": _cg[:, 256:512],
                "g2": _cg[:, 512:768], "b2": _cg[:, 768:1024],
                "epsb": _cg[:, 1024:1025],
                "bint_row": _crow[:, 0:D],
                "bout_row": _crow[:, D:2 * D],
                "ones_col": _crow[:, 2 * D:2 * D + 128],
            }

            from concourse.tile_rust import add_dep_helper
            import contextlib
            loopctx = tc.For_i(0, repeats, 1) if repeats > 1 else contextlib.nullcontext()
            with loopctx:
              st = [dict() for _ in range(BL)]

              def load_emb(b):
                  s = st[b]
                  s["emb"] = embp.tile([128, NT, D], F32R, tag="emb",
                                       name=f"emb_{b}")
                  eap = emb_d[b].rearrange("(t q) d -> q t d", q=128)
                  for k4 in range(4):
                      nc.sync.dma_start(s["emb"][:, 8 * k4:8 * (k4 + 1), :],
                                        eap[:, 8 * k4:8 * (k4 + 1), :])

              def prologue(b):
                  s = st[b]
                  pp3 = prep.tile([3, N], F32, tag="pp3", name=f"pp3_{b}")
                  nc.sync.dma_start(pp3[0:1, :],
                                    pos_d[b, :, :].rearrange("n one -> one n"))
                  nc.sync.dma_start(pp3[1:2, :], cd["crow"][:, 0:N])
                  p16 = prep.tile([16, 256], F32, tag="p16", name=f"p16_{b}")
                  nc.sync.dma_start(p16[:],
                                    pos_d[b, :, 0].rearrange("(k j) -> k j", k=16))
                  q16 = prep.tile([16, 256], F32, tag="q16", name=f"q16_{b}")
                  nc.scalar.square(q16[:], p16[:])
                  iw = nc.sync.dma_start(
                      scratch_d[b].rearrange("(k j) -> k j", k=16), q16[:])
                  ir = nc.sync.dma_start(
                      pp3[2:3, :], scratch_d[b].rearrange("(one n) -> one n", one=1))
                  add_dep_helper(ir.ins, iw.ins, sync=True, reason="scratch RAW")
                  s["pp3"] = pp3

              def stage1(b):
                  s = st[b]
                  pp3, emb_sb = s["pp3"], s["emb"]
                  phiT = [phitp.tile([R, 512], F32R, tag=f"phiT{j}",
                                     name=f"phiT_{b}_{j}") for j in range(NCHUNK)]
                  s["phiT"] = phiT
                  pC = ppA.tile([R, D], F32, tag="Cacc", bufs=2, name=f"pC_{b}")
                  for j in range(NCHUNK):
                      pphi = ppB.tile([R, 512], F32, tag="big", name=f"pphi_{b}_{j}")
                      nc.tensor.matmul(pphi[:], ct["anch"][:, :],
                                       pp3[:, 512 * j:512 * (j + 1)],
                                       start=True, stop=True)
                      nc.scalar.activation(phiT[j][:], pphi[:], ACTF.Exp)
                      for h in range(4):
                          t = 4 * j + h
                          ptr = ppB.tile([128, 128], F32R, tag="tr", bufs=2,
                                         name=f"ptr_{b}_{t}")
                          nc.tensor.transpose(ptr[:],
                                              phiT[j][:, 128 * h:128 * (h + 1)],
                                              ct["ident"][:, :])
                          phiN = wp.tile([128, R], F32R, tag="phiN",
                                         name=f"phiN_{b}_{t}")
                          nc.vector.tensor_copy(phiN[:], ptr[:])
                          nc.tensor.matmul(pC[:], phiN[:], emb_sb[:, t, :],
                                           start=(t == 0), stop=(t == NT - 1))
                  craw = coefp.tile([R, D], F32R, tag="craw", name=f"craw_{b}")
                  nc.scalar.copy(craw[:], pC[:])
                  pC2 = ppB.tile([R, D], F32, tag="mm", bufs=3, name=f"pC2_{b}")
                  nc.tensor.matmul(pC2[:], ct["wq"][:, :], craw[:],
                                   start=True, stop=True)
                  C = coefp.tile([R, D], F32R, tag="C", bufs=4, name=f"C_{b}")
                  nc.scalar.copy(C[:], pC2[:])
                  s["C"] = C

              def diffuse(b):
                  s = st[b]
                  C = s["C"]
                  for step in range(NUM_STEPS):
                      Ct = wp.tile([128, 2, R], F32R, tag="Ct",
                                   name=f"Ct_{b}_{step}")
                      for h in range(2):
                          ptr = ppB.tile([128, 128], F32R, tag="tr", bufs=2,
                                         name=f"ctr_{b}_{step}_{h}")
                          nc.tensor.transpose(ptr[:], C[:, 128 * h:128 * (h + 1)],
                                              ct["ident"][:, :])
                          nc.vector.tensor_copy(Ct[:, h, :], ptr[:])
                      pCW = ppB.tile([R, D], F32, tag="mm", bufs=3,
                                     name=f"pCW_{b}_{step}")
                      for h in range(2):
                          nc.tensor.matmul(pCW[:], Ct[:, h, :], ct["wi"][:, h, :],
                                           start=(h == 0), stop=(h == 1))
                      CW = wp.tile([R, D], F32R, tag="CW", name=f"CW_{b}_{step}")
                      nc.vector.tensor_copy(CW[:], pCW[:])
                      pCn = ppA.tile([R, D], F32, tag="Cacc", bufs=2,
                                     name=f"pCn_{b}_{step}")
                      nc.tensor.matmul(pCn[:], ct["slt"][:, :], C[:, :],
                                       start=True, stop=False)
                      for gt in range(8):
                          pint = ppB.tile([128, D], F32, tag="mm", bufs=3,
                                          name=f"pint_{b}_{step}_{gt}")
                          nc.tensor.matmul(pint[:],
                                           ct["q_sb"][:, 128 * gt:128 * (gt + 1)],
                                           CW[:], start=True,
                                           stop=not flags["use_bint"])
                          if flags["use_bint"]:
                              nc.tensor.matmul(pint[:], ct["ones_col"][:, :],
                                               ct["bint_row"][:, :],
                                               start=False, stop=True)
                          T = wp.tile([128, D], F32R, tag="Ttile",
                                      name=f"T_{b}_{step}_{gt}")
                          nc.scalar.activation(T[:], pint[:], ACTF.Tanh)
                          nc.tensor.matmul(pCn[:], ct["qt_proj"][:, gt, :], T[:],
                                           start=False, stop=(gt == 7))
                      C = coefp.tile([R, D], F32R, tag="C", bufs=4,
                                     name=f"C_{b}_{step}")
                      nc.vector.tensor_copy(C[:], pCn[:])
                  pMC = ppB.tile([R, D], F32, tag="mm", bufs=3, name=f"pMC_{b}")
                  nc.tensor.matmul(pMC[:], ct["mqt"][:, :], C[:],
                                   start=True, stop=True)
                  MC = coefp.tile([R, D], F32R, tag="MC", name=f"MC_{b}")
                  nc.vector.tensor_copy(MC[:], pMC[:])
                  s["MC"] = MC

              def epilogue(b):
                  s = st[b]
                  phiT, MC, emb_sb = s["phiT"], s["MC"], s["emb"]
                  GRP = 6
                  for g0 in range(0, NT, GRP):
                      tl = list(range(g0, min(g0 + GRP, NT)))
                      xs, mv1s, rstds, enhs, enhTs = {}, {}, {}, {}, {}
                      vs, mv2s, rstd2s = {}, {}, {}
                      for t in tl:
                          j, h = divmod(t, 4)
                          psamp = ppB.tile([128, D], F32, tag="mm", bufs=3,
                                           name=f"psamp_{b}_{t}")
                          nc.tensor.matmul(psamp[:],
                                           phiT[j][:, 128 * h:128 * (h + 1)],
                                           MC[:], start=True, stop=False)
                          nc.tensor.matmul(psamp[:], ct["ident"][:, :],
                                           emb_sb[:, t, :], start=False, stop=True)
                          xs[t] = wp.tile([128, D], F32, tag="x", bufs=7,
                                          name=f"x_{b}_{t}")
                          nc.scalar.copy(xs[t][:], psamp[:])
                      for t in tl:
                          bn1 = tp.tile([128, 6], F32, tag="bn1", bufs=8,
                                        name=f"bn1_{b}_{t}")
                          nc.vector.bn_stats(bn1[:], xs[t][:])
                          mv1s[t] = tp.tile([128, 2], F32, tag="mv1", bufs=8,
                                            name=f"mv1_{b}_{t}")
                          nc.vector.bn_aggr(mv1s[t][:], bn1[:])
                      for t in tl:
                          rstds[t] = tp.tile([128, 1], F32, tag="rstd", bufs=8,
                                             name=f"rstd_{b}_{t}")
                          nc.scalar.activation(rstds[t][:], mv1s[t][:, 1:2],
                                               ACTF.Sqrt, bias=ct["epsb"][:, :])
                      for t in tl:
                          nc.vector.reciprocal(rstds[t][:], rstds[t][:])
                      for t in tl:
                          enh = wp.tile([128, D], F32R, tag="enh", bufs=8,
                                        name=f"enh_{b}_{t}")
                          nc.vector.tensor_scalar(enh[:], xs[t][:], mv1s[t][:, 0:1],
                                                  rstds[t][:],
                                                  op0=ALU.subtract, op1=ALU.mult)
                          if flags["ln1_aff"]:
                              enh2 = wp.tile([128, D], F32R, tag="enh2",
                                             name=f"enh2_{b}_{t}")
                              nc.vector.tensor_mul(enh2[:], enh[:].bitcast(F32),
                                                   ct["g1"][:, :])
                              nc.vector.tensor_add(enh2[:], enh2[:].bitcast(F32),
                                                   ct["b1"][:, :])
                              enh = enh2
                          enhs[t] = enh
                      for t in tl:
                          ptr2 = ppB.tile([128, D], F32R, tag="tr", bufs=2,
                                          name=f"ptr2_{b}_{t}")
                          for h2 in range(2):
                              nc.tensor.transpose(ptr2[:, 128 * h2:128 * (h2 + 1)],
                                                  enhs[t][:, 128 * h2:128 * (h2 + 1)],
                                                  ct["ident"][:, :])
                          enhTs[t] = wp.tile([128, 2, 128], F32R, tag="enhT", bufs=6,
                                             name=f"enhT_{b}_{t}")
                          nc.scalar.copy(enhTs[t][:].rearrange("p a b -> p (a b)"),
                                         ptr2[:])
                      for t in tl:
                          pout1 = ppB.tile([128, D], F32, tag="mm", bufs=3,
                                           name=f"pout1_{b}_{t}")
                          for h2 in range(2):
                              nc.tensor.matmul(pout1[:], enhTs[t][:, h2, :],
                                               ct["wo"][:, h2, :],
                                               start=(h2 == 0), stop=False)
                          if flags["use_bout"]:
                              nc.tensor.matmul(pout1[:], ct["ones_col"][:, :],
                                               ct["bout_row"][:, :],
                                               start=False, stop=False)
                          nc.tensor.matmul(pout1[:], ct["ident"][:, :], enhs[t][:],
                                           start=False, stop=True)
                          vs[t] = wp.tile([128, D], F32, tag="v", bufs=7,
                                          name=f"v_{b}_{t}")
                          nc.scalar.copy(vs[t][:], pout1[:])
                      for t in tl:
                          bn2 = tp.tile([128, 6], F32, tag="bn2", bufs=8,
                                        name=f"bn2_{b}_{t}")
                          nc.vector.bn_stats(bn2[:], vs[t][:])
                          mv2s[t] = tp.tile([128, 2], F32, tag="mv2", bufs=8,
                                            name=f"mv2_{b}_{t}")
                          nc.vector.bn_aggr(mv2s[t][:], bn2[:])
                      for t in tl:
                          rstd2s[t] = tp.tile([128, 1], F32, tag="rstd2", bufs=8,
                                              name=f"rstd2_{b}_{t}")
                          nc.scalar.activation(rstd2s[t][:], mv2s[t][:, 1:2],
                                               ACTF.Sqrt, bias=ct["epsb"][:, :])
                      for t in tl:
                          nc.vector.reciprocal(rstd2s[t][:], rstd2s[t][:])
                      ot8 = None
                      for t in tl:
                          if t % 2 == 0:
                              ot8 = wp.tile([128, 2, D], F32, tag="ot8", bufs=3,
                                            name=f"ot8_{b}_{t}")
                          nc.vector.tensor_scalar(ot8[:, t % 2, :], vs[t][:],
                                                  mv2s[t][:, 0:1], rstd2s[t][:],
                                                  op0=ALU.subtract, op1=ALU.mult)
                          if flags["ln2_aff"]:
                              nc.vector.tensor_mul(ot8[:, t % 2, :],
                                                   ot8[:, t % 2, :], ct["g2"][:, :])
                              nc.vector.tensor_add(ot8[:, t % 2, :],
                                                   ot8[:, t % 2, :], ct["b2"][:, :])
                          if t % 2 == 1:
                              g8 = t // 2
                              nc.sync.dma_start(
                                  out_d[b].rearrange("(t q) d -> q t d", q=128)
                                       [:, 2 * g8:2 * (g8 + 1), :],
                                  ot8[:])

              # phase-grouped emission: both batches interleave per phase
              for b in range(BL):
                  prologue(b)
              for b in range(BL):
                  load_emb(b)
              if "s1" in parts:
                  for b in range(BL):
                      stage1(b)
                  if "diff" in parts:
                      for b in range(BL):
                          diffuse(b)
                  else:
                      for b in range(BL):
                          st[b]["MC"] = st[b]["C"]
                  if "epi" in parts:
                      for b in range(BL):
                          epilogue(b)

    nc.compile()
    return nc


# --------------------------------------------------------------------------
# runner (compiled-callable cache; replicates bass2jax.run_bass_via_pjrt's
# multi-core path but keeps the jitted function so repeat calls don't relower)
# --------------------------------------------------------------------------
def _make_runner(nc):
    import jax
    import numpy as _np
    from jax.sharding import Mesh, PartitionSpec
    from jax.experimental.shard_map import shard_map
    from concourse import mybir as _mb
    from concourse.bass2jax import (install_neuronx_cc_hook, _bass_exec_p,
                                    partition_id_tensor)
    install_neuronx_cc_hook()
    partition_name = nc.partition_id_tensor.name if nc.partition_id_tensor else None
    in_names, out_names, out_avals, zero_outs = [], [], [], []
    for alloc in nc.m.functions[0].allocations:
        if not isinstance(alloc, _mb.MemoryLocationSet):
            continue
        name = alloc.memorylocations[0].name
        if alloc.kind == "ExternalInput":
            if name != partition_name:
                in_names.append(name)
        elif alloc.kind == "ExternalOutput":
            npdt = _mb.dt.np(alloc.dtype)
            out_names.append(name)
            out_avals.append(jax.core.ShapedArray(tuple(alloc.tensor_shape), npdt))
            zero_outs.append(_np.zeros(tuple(alloc.tensor_shape), npdt))
    n_params = len(in_names)
    n_outs = len(out_names)
    all_in = in_names + out_names + ([partition_name] if partition_name else [])

    def _body(*args):
        operands = list(args)
        if partition_name is not None:
            operands.append(partition_id_tensor())
        return tuple(_bass_exec_p.bind(
            *operands, out_avals=tuple(out_avals),
            in_names=tuple(all_in), out_names=tuple(out_names),
            lowering_input_output_aliases=(), sim_require_finite=True,
            sim_require_nnan=True, nc=nc))

    devices = jax.devices()[:NCORES]
    mesh = Mesh(_np.asarray(devices), ("core",))
    donate = tuple(range(n_params, n_params + n_outs))
    sharded = jax.jit(
        shard_map(_body, mesh=mesh,
                  in_specs=(PartitionSpec("core"),) * (n_params + n_outs),
                  out_specs=(PartitionSpec("core"),) * n_outs,
                  check_rep=False),
        donate_argnums=donate, keep_unused=True)

    def run(in_maps):
        per_core = [[_np.asarray(m[name]) for name in in_names] for m in in_maps]
        concat_in = [_np.concatenate([per_core[c][i] for c in range(NCORES)], axis=0)
                     for i in range(n_params)]
        concat_zero = [_np.zeros((NCORES * z.shape[0], *z.shape[1:]), z.dtype)
                       for z in zero_outs]
        outs = sharded(*concat_in, *concat_zero)
        outs = [_np.asarray(o) for o in outs]
        return {name: outs[i] for i, name in enumerate(out_names)}

    return run


def kernel(**inputs):
    emb = np.ascontiguousarray(inputs["embeddings"], dtype=np.float32)
    pos = np.ascontiguousarray(inputs["positions"], dtype=np.float32)
    grid = np.asarray(inputs["grid_points"], dtype=np.float64)[0, :, 0]
    params = dict(
        sigma=float(np.asarray(inputs["sigma"])),
        alpha=float(np.asarray(inputs["alpha"])),
        grid=grid,
        W_int=np.asarray(inputs["W_int"], np.float64),
        b_int=np.asarray(inputs["b_int"], np.float64),
        W_out=np.asarray(inputs["W_out"], np.float64),
        b_out=np.asarray(inputs["b_out"], np.float64),
        ln1_g=np.asarray(inputs["ln1_g"], np.float64),
        ln1_b=np.asarray(inputs["ln1_b"], np.float64),
        ln2_g=np.asarray(inputs["ln2_g"], np.float64),
        ln2_b=np.asarray(inputs["ln2_b"], np.float64),
    )
    key = hashlib.sha256(b"".join(np.asarray(v).tobytes() for v in params.values())).hexdigest()
    if key not in _CACHE:
        consts, flags = _host_plan(**params)
        nc = _build_module(flags)
        _CACHE[key] = (_make_runner(nc), consts)
    run, consts = _CACHE[key]

    in_maps = []
    for c in range(NCORES):
        m = {"emb": emb[BL * c:BL * (c + 1)],
             "pos": pos[BL * c:BL * (c + 1)]}
        m.update(consts)
        in_maps.append(m)
    outs = run(in_maps)
    # outs["out"] is [NCORES*BL, N, D] concatenated over cores
    return np.ascontiguousarray(outs["out"], dtype=np.float32)



# revision 23
# speedup vs baseline: 1.6821x; 1.6821x over previous
"""Trainium2 Bass kernel for nn_EnhancedTFNLayer (RBF field projection +
diffusion + sampling + LN/linear epilogue), data-parallel over batch on 8 cores.

Low-rank field pipeline (host-fitted operators, rank R=128):

  phi[n, j] = exp(-(p_n - c_j)^2 / (2 s^2))          anchor features
     K=8 split-feature f32r matmul (exact tf32 products) + Act Exp
  C_raw = phi^T emb;  C0 = Wq^T C_raw                 Q-coordinates of field
  single fused diffusion step (validated ~2e-6 vs 4-step reference):
     C4 = SL^4 C0 + [DT * sum_k SL^k Q Lint] tanh(Qc^T (C0 W_int))
     (tanh evaluated at 256 coarse grid points, linear-interp operator Lint)
  MC = MQ C4;  sampled = phi MC
  x = sampled + emb;  enh = LN1(x)  (bn_stats from PSUM)
  v = enh (W_out + I) (+ folded affine/bias rank-1)
  out = LN2(v)

Both batches of a core are paired into [128, 512] tiles everywhere past the
projection. All bulk tensors bf16 (validated 3e-3 rel err vs 2e-2 budget).
"""
import sys
import hashlib
import numpy as np
import ml_dtypes

for _p in ("/opt/trn_rl_repo", "/root/.axon_site/_ro/trn_rl_repo"):
    if _p not in sys.path:
        sys.path.insert(0, _p)

import concourse.bass as bass
import concourse.bacc as bacc
import concourse.tile as tile
from concourse import mybir

F32 = mybir.dt.float32
F32R = mybir.dt.float32r
BF16 = mybir.dt.bfloat16
ACTF = mybir.ActivationFunctionType
ALU = mybir.AluOpType

B, N, G, D = 16, 4096, 1024, 256
NUM_STEPS, DT, EPS = 4, 0.01, 1e-5
R = 128
GP = 256                 # coarse grid for tanh evaluation
NT = N // 128            # 32 token tiles per batch
BL = 2                   # batches per core
NCORES = 8

_CACHE = {}

BF = ml_dtypes.bfloat16


def _tf32(x):
    x32 = np.asarray(x, np.float32)
    u = x32.view(np.uint32)
    u = (u + np.uint32(0x1000)) & np.uint32(0xFFFFE000)
    return u.view(np.float32)


def _bf(x):
    return np.ascontiguousarray(np.asarray(x, np.float32).astype(BF))


# --------------------------------------------------------------------------
# host-side operator fitting (float64; parameter inputs only)
# --------------------------------------------------------------------------
def _host_plan(sigma, alpha, grid, W_int, b_int, W_out, b_out,
               ln1_g, ln1_b, ln2_g, ln2_b):
    rng = np.random.default_rng(0)
    c0 = 1.0 - 2.0 * alpha * DT
    c1 = alpha * DT
    pg = np.linspace(0.0, 1.0, 8193)
    K = np.exp(-((pg[:, None] - grid[None, :]) ** 2) / (2 * sigma * sigma))
    # basis enrichment with synthetic tanh fields (params only, no data)
    nsyn = 384
    sub = rng.choice(len(pg), size=256, replace=False)
    Fsyn = K[sub].T @ rng.standard_normal((256, nsyn))
    Fsyn /= np.abs(Fsyn).max(0, keepdims=True) + 1e-30
    fscale = np.sqrt(N * sigma * np.sqrt(np.pi))
    wnorm = np.linalg.norm(W_int, axis=0)
    wcols = rng.choice(len(wnorm), size=nsyn)
    gains = fscale * wnorm[wcols] * rng.uniform(0.5, 2.0, nsyn)
    Tsyn = np.tanh(Fsyn * gains[None, :])
    Msvd = np.concatenate([K, (Tsyn * 0.1).T], axis=0)
    _, _, Vt = np.linalg.svd(Msvd, full_matrices=False)
    Q = Vt[:R]                                            # [R, G]
    # anchors
    c = np.linspace(-0.08, 1.08, R)
    s = 2.2 * (c[1] - c[0])
    F = np.exp(-((pg[:, None] - c[None, :]) ** 2) / (2 * s * s))
    Qk = K @ Q.T
    Wq, *_ = np.linalg.lstsq(F, Qk, rcond=1e-8)           # [R, R]
    # diffusion operator in Q coords (exact edge-padded 3-tap applied to Q^T)
    Qt = Q.T
    LQt = c0 * Qt.copy()
    LQt[1:-1] += c1 * (Qt[:-2] + Qt[2:])
    LQt[0] += c1 * (Qt[0] + Qt[1])
    LQt[-1] += c1 * (Qt[-2] + Qt[-1])
    SLQ = Q @ LQt                                         # [R, R]
    # single fused step: C4 = SLQ^4 C0 + DT (I+SLQ+SLQ^2+SLQ^3) Q Lint T0
    Afold = np.linalg.matrix_power(SLQ, 4)
    Ssum = np.eye(R) + SLQ + SLQ @ SLQ + SLQ @ SLQ @ SLQ
    stride = G // GP
    Qc = Q[:, ::stride]                                   # [R, GP]
    Lint = np.zeros((G, GP))
    for g in range(G):
        x = g / stride
        j = min(int(np.floor(x)), GP - 2)
        t = x - j
        Lint[g, j] = 1 - t
        Lint[g, j + 1] = t
    Ptot = DT * (Ssum @ (Q @ Lint))                       # [R, GP]
    # sampling (linear interp of Q columns) fitted over anchors
    u = pg * (G - 1)
    i0 = np.clip(np.floor(u), 0, G - 2).astype(int)
    w = u - i0
    lerpQ = Qt[i0] * (1 - w)[:, None] + Qt[i0 + 1] * w[:, None]
    MQ, *_ = np.linalg.lstsq(F, lerpQ, rcond=1e-5)        # [R, R]

    # epilogue fold: v = enh' W_out + b_out + enh' with enh' = enh*g1 + b1
    #   => v = enh Wp + r0,  Wp = diag(g1)(W_out + I),  r0 = b1(W_out+I)+b_out
    Wp = np.diag(ln1_g) @ (W_out + np.eye(D))
    r0 = ln1_b @ (W_out + np.eye(D)) + b_out

    # phi exponent split features: arg = a1*p + a2 - k*p^2, all products
    # computed exactly in f32r via hi/lo splits (validated 3e-5 rel err)
    k_ = 1.0 / (2 * s * s)
    a1 = 2 * k_ * c
    a2 = -k_ * c * c
    a1_hi = _tf32(a1).astype(np.float64)
    a1_lo = _tf32(a1 - a1_hi)
    a2_hi = _tf32(a2).astype(np.float64)
    a2_lo = _tf32(a2 - a2_hi)
    kv = np.full(R, k_)
    k_hi = _tf32(kv).astype(np.float64)
    k_lo = _tf32(kv - k_hi)
    anch8 = np.stack([a1_hi.astype(np.float32), a1_lo, a1_hi.astype(np.float32),
                      -k_hi.astype(np.float32), -k_lo, -k_hi.astype(np.float32),
                      a2_hi.astype(np.float32), a2_lo])   # [8, R]

    # f32r const blob [128, 1280] (C-coefficient chain needs > bf16 precision):
    # wq | afold_t | mqt | qc(256) | wi(2x256) | identr
    blk = lambda M: M.reshape(2, 128, -1).transpose(1, 0, 2).reshape(128, -1)
    crf = np.concatenate([
        Wq,                        # lhsT: C0 = Wq^T Craw
        Afold.T,                   # lhsT: Afold C0
        MQ.T,                      # lhsT: MQ C4
        Qc,                        # lhsT blocks: Qc^T CW
        blk(W_int),                # rhs halves
        np.eye(128),
    ], axis=1)
    # bf16 const blob [128, 896]: ptot_t(2x128) | wp(2x256) | identb
    crb = np.concatenate([
        blk(Ptot.T),               # lhsT blocks: Ptot T0
        blk(Wp),                   # rhs halves
        np.eye(128),
    ], axis=1)
    crow = np.concatenate([np.ones((1, 128)), b_int.reshape(1, D),
                           r0.reshape(1, D)], axis=1)     # [1, 640]
    consts = {
        "anch8": np.ascontiguousarray(anch8, np.float32),
        "crf": np.ascontiguousarray(crf, np.float32),
        "crb": _bf(crb),
        "crow": _bf(crow),
    }
    flags = {
        "use_bint": bool(np.any(b_int != 0)),
        "use_r0": bool(np.any(r0 != 0)),
        "ln2_aff": bool(np.any(ln2_g != 1) or np.any(ln2_b != 0)),
        "ln2_vals": (np.asarray(ln2_g, np.float64), np.asarray(ln2_b, np.float64)),
    }
    return consts, flags


def _make_pp8(pos):
    """pos [B?, N] f32 -> [B?, 8, N] f32 split-feature rows (tf32-exact)."""
    p32 = np.asarray(pos, np.float32)
    p_hi = _tf32(p32)
    p_lo = _tf32(p32.astype(np.float64) - p_hi)
    p2 = (p32.astype(np.float64) ** 2).astype(np.float32)
    p2_hi = _tf32(p2)
    p2_lo = _tf32(p2.astype(np.float64) - p2_hi)
    ones = np.ones_like(p32)
    return np.ascontiguousarray(
        np.stack([p_hi, p_hi, p_lo, p2_hi, p2_hi, p2_lo, ones, ones], axis=-2))


# --------------------------------------------------------------------------
# device module
# --------------------------------------------------------------------------
def _build_module(flags, repeats=1, parts=("s1", "diff", "epi")):
    nc = bacc.Bacc(trn_type="TRN2")
    emb_d = nc.dram_tensor("emb", [BL, N, D], BF16, kind="ExternalInput")
    pp8_d = nc.dram_tensor("pp8", [BL, 8, N], F32R, kind="ExternalInput")
    anch_d = nc.dram_tensor("anch8", [8, R], F32R, kind="ExternalInput")
    crf_d = nc.dram_tensor("crf", [128, 1280], F32R, kind="ExternalInput")
    crb_d = nc.dram_tensor("crb", [128, 896], BF16, kind="ExternalInput")
    crow_d = nc.dram_tensor("crow", [1, 640], BF16, kind="ExternalInput")
    out_d = nc.dram_tensor("out", [BL, N, D], BF16, kind="ExternalOutput")

    with tile.TileContext(nc) as tc:
        with tc.tile_pool(name="consts", bufs=1) as cp, \
             tc.tile_pool(name="emb", bufs=1) as embp, \
             tc.tile_pool(name="phit", bufs=1) as phitp, \
             tc.tile_pool(name="phin", bufs=1) as phinp, \
             tc.tile_pool(name="coef", bufs=2) as coefp, \
             tc.tile_pool(name="work", bufs=4) as wkp, \
             tc.tile_pool(name="tiny", bufs=8) as tp, \
             tc.tile_pool(name="ppA", bufs=1, space="PSUM") as ppA, \
             tc.tile_pool(name="ppB", bufs=1, space="PSUM") as ppB, \
             tc.tile_pool(name="ppT", bufs=1, space="PSUM") as ppT:

            # ---- constants ----
            anch_sb = cp.tile([8, R], F32R, tag="anch8", name="c_anch8")
            nc.sync.dma_start(anch_sb[:], anch_d[:, :])
            pp8_sb = [cp.tile([8, N], F32R, tag=f"pp8_{b}", name=f"c_pp8_{b}")
                      for b in range(BL)]
            for b in range(BL):
                nc.sync.dma_start(pp8_sb[b][:], pp8_d[b])
            crf = cp.tile([128, 1280], F32R, tag="crf", name="c_crf")
            nc.sync.dma_start(crf[:], crf_d[:, :])
            crb = cp.tile([128, 896], BF16, tag="crb", name="c_crb")
            nc.sync.dma_start(crb[:], crb_d[:, :])
            crow = cp.tile([1, 640], BF16, tag="crow", name="c_crow")
            nc.sync.dma_start(crow[:], crow_d[:, :])
            ct = {
                "wq": crf[:, 0:128], "afold": crf[:, 128:256],
                "mqt": crf[:, 256:384], "qc": crf[:, 384:640],
                "wi": crf[:, 640:1152].rearrange("p (a b) -> p a b", a=2),
                "identr": crf[:, 1152:1280],
                "ptot": crb[:, 0:256].rearrange("p (a b) -> p a b", a=2),
                "wp": crb[:, 256:768].rearrange("p (a b) -> p a b", a=2),
                "ident": crb[:, 768:896],
                "ones_col": crow[:, 0:128],
                "bint_row": crow[:, 128:384],
                "r0_row": crow[:, 384:640],
            }
            epsb = cp.tile([128, 1], F32, tag="epsb", name="c_epsb")
            nc.vector.memset(epsb[:], EPS)

            # emb resident: [128, b, t, d]
            emb_sb = embp.tile([128, BL, NT, D], BF16, tag="emb", name="emb_sb")
            for b in range(BL):
                eap = emb_d[b].rearrange("(t q) d -> q t d", q=128)
                for k4 in range(4):
                    nc.sync.dma_start(emb_sb[:, b, 8 * k4:8 * (k4 + 1), :],
                                      eap[:, 8 * k4:8 * (k4 + 1), :])

            import contextlib
            loopctx = tc.For_i(0, repeats, 1) if repeats > 1 else contextlib.nullcontext()
            with loopctx:
              phiT = [phitp.tile([R, 8, 512], BF16, tag=f"phiT{b}",
                                 name=f"phiT_{b}") for b in range(BL)]
              phiN = [phinp.tile([128, NT, R], BF16, tag=f"phiN{b}",
                                 name=f"phiN_{b}") for b in range(BL)]

              # ---- stage 1: phi (R-major + token-major) and C fold ----
              pC2 = ppA.tile([R, BL * D], F32, tag="Cacc", name="pC2")
              for b in range(BL):
                  for j in range(8):
                      pphi = ppB.tile([R, 512], F32, tag="mm", bufs=2,
                                      name=f"pphi_{b}_{j}")
                      nc.tensor.matmul(pphi[:], anch_sb[:, :],
                                       pp8_sb[b][:, 512 * j:512 * (j + 1)],
                                       start=True, stop=True)
                      nc.scalar.activation(phiT[b][:, j, :], pphi[:], ACTF.Exp)
                      ptrN = ppT.tile([128, 512], BF16, tag="tr", bufs=2,
                                      name=f"ptrN_{b}_{j}")
                      for h in range(4):
                          nc.tensor.transpose(ptrN[:, 128 * h:128 * (h + 1)],
                                              phiT[b][:, j, 128 * h:128 * (h + 1)],
                                              ct["ident"][:, :])
                      eng = nc.scalar if (j % 2 == 0) else nc.vector
                      if eng is nc.scalar:
                          nc.scalar.copy(
                              phiN[b][:, 4 * j:4 * (j + 1), :],
                              ptrN[:].rearrange("p (a b) -> p a b", a=4))
                      else:
                          nc.vector.tensor_copy(
                              phiN[b][:, 4 * j:4 * (j + 1), :],
                              ptrN[:].rearrange("p (a b) -> p a b", a=4))
              if "s1" in parts:
                  for b in range(BL):
                      for t in range(NT):
                          nc.tensor.matmul(pC2[:, D * b:D * (b + 1)],
                                           phiN[b][:, t, :], emb_sb[:, b, t, :],
                                           start=(t == 0), stop=(t == NT - 1))

              # ---- fused diffusion (paired batches, [R, 512] tiles) ----
              def diffuse():
                  craw2 = coefp.tile([R, 512], F32R, tag="craw2", name="craw2")
                  nc.scalar.copy(craw2[:], pC2[:])
                  pC0 = ppB.tile([R, 512], F32, tag="mm", bufs=2, name="pC0")
                  nc.tensor.matmul(pC0[:], ct["wq"][:, :], craw2[:],
                                   start=True, stop=True)
                  C02 = coefp.tile([R, 512], F32R, tag="C02", name="C02")
                  nc.scalar.copy(C02[:], pC0[:])
                  ptrC = ppB.tile([128, 512], F32R, tag="trC", bufs=1, name="ptrC")
                  for h in range(4):
                      nc.tensor.transpose(ptrC[:, 128 * h:128 * (h + 1)],
                                          C02[:, 128 * h:128 * (h + 1)],
                                          ct["identr"][:, :])
                  ctC = coefp.tile([128, 4, 128], F32R, tag="ctC", name="ctC")
                  nc.vector.tensor_copy(
                      ctC[:], ptrC[:].rearrange("p (a b) -> p a b", a=4))
                  pCW = ppB.tile([R, 512], F32, tag="mm", bufs=2, name="pCW")
                  for b in range(BL):
                      for h in range(2):
                          nc.tensor.matmul(pCW[:, D * b:D * (b + 1)],
                                           ctC[:, 2 * b + h, :], ct["wi"][:, h, :],
                                           start=(h == 0), stop=(h == 1))
                  CW2 = coefp.tile([R, 512], F32R, tag="CW2", name="CW2")
                  nc.scalar.copy(CW2[:], pCW[:])
                  Tb = []
                  for blk in range(2):
                      pT = ppB.tile([128, 512], F32, tag="mm", bufs=2,
                                    name=f"pT_{blk}")
                      for b in range(BL):
                          nc.tensor.matmul(pT[:, D * b:D * (b + 1)],
                                           ct["qc"][:, 128 * blk:128 * (blk + 1)],
                                           CW2[:, D * b:D * (b + 1)],
                                           start=True,
                                           stop=not flags["use_bint"])
                          if flags["use_bint"]:
                              nc.tensor.matmul(pT[:, D * b:D * (b + 1)],
                                               ct["ones_col"][:, :],
                                               ct["bint_row"][:, :],
                                               start=False, stop=True)
                      T_sb = coefp.tile([128, 512], BF16, tag=f"T{blk}",
                                        name=f"T_{blk}")
                      nc.scalar.activation(T_sb[:], pT[:], ACTF.Tanh)
                      Tb.append(T_sb)
                  pC4 = ppB.tile([R, 512], F32, tag="mm", bufs=2, name="pC4")
                  for b in range(BL):
                      nc.tensor.matmul(pC4[:, D * b:D * (b + 1)],
                                       ct["afold"][:, :], C02[:, D * b:D * (b + 1)],
                                       start=True, stop=False)
                      for blk in range(2):
                          nc.tensor.matmul(pC4[:, D * b:D * (b + 1)],
                                           ct["ptot"][:, blk, :],
                                           Tb[blk][:, D * b:D * (b + 1)],
                                           start=False,
                                           stop=(blk == 1))
                  C42 = coefp.tile([R, 512], F32R, tag="C42", name="C42")
                  nc.scalar.copy(C42[:], pC4[:])
                  pMC = ppB.tile([R, 512], F32, tag="mm", bufs=2, name="pMC")
                  nc.tensor.matmul(pMC[:], ct["mqt"][:, :], C42[:],
                                   start=True, stop=True)
                  MC2 = coefp.tile([R, 512], BF16, tag="MC2", name="MC2")
                  nc.scalar.copy(MC2[:], pMC[:])
                  return MC2

              # ---- epilogue: 32 paired supertiles [128 tok, 2 batches, 256] ----
              # Engine split (GPSIMD cannot read PSUM):
              #   Act: x/v/enhT PSUM->SBUF bf16 copies, sqrt
              #   DVE: bn_stats/bn_aggr (SBUF bf16), reciprocal, out TSP halves (4x)
              #   Pool: enh TSP halves (SBUF->SBUF)
              def epilogue(MC2):
                  GRP = 4
                  for g0 in range(0, NT, GRP):
                      tl = list(range(g0, min(g0 + GRP, NT)))
                      ps, xb, mv1, rs1 = {}, {}, {}, {}
                      enh, enhT, po, vb = {}, {}, {}, {}
                      mv2, rs2 = {}, {}
                      for t in tl:
                          j, h = divmod(t, 4)
                          p = ppA.tile([128, 512], F32, tag="samp", bufs=2,
                                       name=f"psamp_{t}")
                          for b in range(BL):
                              nc.tensor.matmul(
                                  p[:, D * b:D * (b + 1)],
                                  phiT[b][:, j, 128 * h:128 * (h + 1)],
                                  MC2[:, D * b:D * (b + 1)],
                                  start=True, stop=False)
                              nc.tensor.matmul(p[:, D * b:D * (b + 1)],
                                               ct["ident"][:, :],
                                               emb_sb[:, b, t, :],
                                               start=False, stop=True)
                          ps[t] = p
                      for t in tl:
                          xb[t] = wkp.tile([128, 512], BF16, tag="xb", bufs=6,
                                           name=f"xb_{t}")
                          nc.scalar.copy(xb[t][:], ps[t][:])
                      for t in tl:
                          bn = tp.tile([128, 2, 6], F32, tag="bn1", bufs=8,
                                       name=f"bn1_{t}")
                          mv1[t] = tp.tile([128, 2, 2], F32, tag="mv1", bufs=8,
                                           name=f"mv1_{t}")
                          for hh in range(2):
                              nc.vector.bn_stats(bn[:, hh, :],
                                                 xb[t][:, D * hh:D * (hh + 1)])
                              nc.vector.bn_aggr(mv1[t][:, hh, :], bn[:, hh, :])
                      for t in tl:
                          rs1[t] = tp.tile([128, 2], F32, tag="rs1", bufs=8,
                                           name=f"rs1_{t}")
                          nc.scalar.activation(
                              rs1[t][:].rearrange("p (a b) -> p a b", b=1),
                              mv1[t][:, :, 1:2], ACTF.Sqrt, bias=epsb[:, :])
                      for t in tl:
                          nc.vector.reciprocal(rs1[t][:], rs1[t][:])
                      for t in tl:
                          e = wkp.tile([128, 512], BF16, tag="enh", bufs=6,
                                       name=f"enh_{t}")
                          for hh in range(2):
                              nc.gpsimd.tensor_scalar(
                                  e[:, D * hh:D * (hh + 1)],
                                  xb[t][:, D * hh:D * (hh + 1)],
                                  mv1[t][:, hh, 0:1], rs1[t][:, hh:hh + 1],
                                  op0=ALU.subtract, op1=ALU.mult)
                          enh[t] = e
                      for t in tl:
                          ptr = ppT.tile([128, 512], BF16, tag="tr", bufs=2,
                                         name=f"ptr2_{t}")
                          for hh in range(4):
                              nc.tensor.transpose(
                                  ptr[:, 128 * hh:128 * (hh + 1)],
                                  enh[t][:, 128 * hh:128 * (hh + 1)],
                                  ct["ident"][:, :])
                          eT = wkp.tile([128, 4, 128], BF16, tag="enhT", bufs=6,
                                        name=f"enhT_{t}")
                          nc.scalar.copy(eT[:],
                                         ptr[:].rearrange("p (a b) -> p a b", a=4))
                          enhT[t] = eT
                      for t in tl:
                          p = ppB.tile([128, 512], F32, tag="mm", bufs=2,
                                       name=f"pout_{t}")
                          for b in range(BL):
                              for hh in range(2):
                                  nc.tensor.matmul(p[:, D * b:D * (b + 1)],
                                                   enhT[t][:, 2 * b + hh, :],
                                                   ct["wp"][:, hh, :],
                                                   start=(hh == 0),
                                                   stop=(hh == 1 and
                                                         not flags["use_r0"]))
                              if flags["use_r0"]:
                                  nc.tensor.matmul(p[:, D * b:D * (b + 1)],
                                                   ct["ones_col"][:, :],
                                                   ct["r0_row"][:, :],
                                                   start=False, stop=True)
                          po[t] = p
                      for t in tl:
                          vb[t] = wkp.tile([128, 512], BF16, tag="vb", bufs=6,
                                           name=f"vb_{t}")
                          nc.scalar.copy(vb[t][:], po[t][:])
                      for t in tl:
                          bn = tp.tile([128, 2, 6], F32, tag="bn2", bufs=8,
                                       name=f"bn2_{t}")
                          mv2[t] = tp.tile([128, 2, 2], F32, tag="mv2", bufs=8,
                                           name=f"mv2_{t}")
                          for hh in range(2):
                              nc.vector.bn_stats(bn[:, hh, :],
                                                 vb[t][:, D * hh:D * (hh + 1)])
                              nc.vector.bn_aggr(mv2[t][:, hh, :], bn[:, hh, :])
                      for t in tl:
                          rs2[t] = tp.tile([128, 2], F32, tag="rs2", bufs=8,
                                           name=f"rs2_{t}")
                          nc.scalar.activation(
                              rs2[t][:].rearrange("p (a b) -> p a b", b=1),
                              mv2[t][:, :, 1:2], ACTF.Sqrt, bias=epsb[:, :])
                      for t in tl:
                          nc.vector.reciprocal(rs2[t][:], rs2[t][:])
                      for t in tl:
                          ot = wkp.tile([128, BL, D], BF16, tag="ot", bufs=4,
                                        name=f"ot_{t}")
                          nc.vector.tensor_scalar(
                              ot[:, 0, :], vb[t][:, 0:256],
                              mv2[t][:, 0, 0:1], rs2[t][:, 0:1],
                              op0=ALU.subtract, op1=ALU.mult)
                          nc.gpsimd.tensor_scalar(
                              ot[:, 1, :], vb[t][:, 256:512],
                              mv2[t][:, 1, 0:1], rs2[t][:, 1:2],
                              op0=ALU.subtract, op1=ALU.mult)
                          nc.sync.dma_start(
                              out_d.rearrange("b (t q) d -> q b t d", q=128)
                                   [:, :, t, :],
                              ot[:])

              if "s1" in parts and "diff" in parts:
                  MC2 = diffuse()
                  if "epi" in parts:
                      epilogue(MC2)

    nc.compile()
    return nc


# --------------------------------------------------------------------------
# runner (compiled-callable cache; replicates bass2jax.run_bass_via_pjrt's
# multi-core path but keeps the jitted function so repeat calls don't relower)
# --------------------------------------------------------------------------
def _make_runner(nc):
    import jax
    import numpy as _np
    from jax.sharding import Mesh, PartitionSpec
    from jax.experimental.shard_map import shard_map
    from concourse import mybir as _mb
    from concourse.bass2jax import (install_neuronx_cc_hook, _bass_exec_p,
                                    partition_id_tensor)
    install_neuronx_cc_hook()
    partition_name = nc.partition_id_tensor.name if nc.partition_id_tensor else None
    in_names, out_names, out_avals, zero_outs = [], [], [], []
    for alloc in nc.m.functions[0].allocations:
        if not isinstance(alloc, _mb.MemoryLocationSet):
            continue
        name = alloc.memorylocations[0].name
        if alloc.kind == "ExternalInput":
            if name != partition_name:
                in_names.append(name)
        elif alloc.kind == "ExternalOutput":
            npdt = _mb.dt.np(alloc.dtype)
            out_names.append(name)
            out_avals.append(jax.core.ShapedArray(tuple(alloc.tensor_shape), npdt))
            zero_outs.append(_np.zeros(tuple(alloc.tensor_shape), npdt))
    n_params = len(in_names)
    n_outs = len(out_names)
    all_in = in_names + out_names + ([partition_name] if partition_name else [])

    def _body(*args):
        operands = list(args)
        if partition_name is not None:
            operands.append(partition_id_tensor())
        return tuple(_bass_exec_p.bind(
            *operands, out_avals=tuple(out_avals),
            in_names=tuple(all_in), out_names=tuple(out_names),
            lowering_input_output_aliases=(), sim_require_finite=True,
            sim_require_nnan=True, nc=nc))

    devices = jax.devices()[:NCORES]
    mesh = Mesh(_np.asarray(devices), ("core",))
    donate = tuple(range(n_params, n_params + n_outs))
    sharded = jax.jit(
        shard_map(_body, mesh=mesh,
                  in_specs=(PartitionSpec("core"),) * (n_params + n_outs),
                  out_specs=(PartitionSpec("core"),) * n_outs,
                  check_rep=False),
        donate_argnums=donate, keep_unused=True)

    def run(in_maps):
        per_core = [[_np.asarray(m[name]) for name in in_names] for m in in_maps]
        concat_in = [_np.concatenate([per_core[c][i] for c in range(NCORES)], axis=0)
                     for i in range(n_params)]
        concat_zero = [_np.zeros((NCORES * z.shape[0], *z.shape[1:]), z.dtype)
                       for z in zero_outs]
        outs = sharded(*concat_in, *concat_zero)
        outs = [_np.asarray(o) for o in outs]
        return {name: outs[i] for i, name in enumerate(out_names)}

    return run


def _core_inputs(emb, pos, consts):
    """Full-batch emb [B,N,D] f32, pos [B,N] f32 -> list of per-core maps."""
    embb = _bf(emb)
    pp8 = _make_pp8(pos)
    in_maps = []
    for c in range(NCORES):
        m = {"emb": embb[BL * c:BL * (c + 1)],
             "pp8": pp8[BL * c:BL * (c + 1)]}
        m.update(consts)
        in_maps.append(m)
    return in_maps


def kernel(**inputs):
    emb = np.ascontiguousarray(inputs["embeddings"], dtype=np.float32)
    pos = np.ascontiguousarray(inputs["positions"], dtype=np.float32)[..., 0]
    grid = np.asarray(inputs["grid_points"], dtype=np.float64)[0, :, 0]
    params = dict(
        sigma=float(np.asarray(inputs["sigma"])),
        alpha=float(np.asarray(inputs["alpha"])),
        grid=grid,
        W_int=np.asarray(inputs["W_int"], np.float64),
        b_int=np.asarray(inputs["b_int"], np.float64),
        W_out=np.asarray(inputs["W_out"], np.float64),
        b_out=np.asarray(inputs["b_out"], np.float64),
        ln1_g=np.asarray(inputs["ln1_g"], np.float64),
        ln1_b=np.asarray(inputs["ln1_b"], np.float64),
        ln2_g=np.asarray(inputs["ln2_g"], np.float64),
        ln2_b=np.asarray(inputs["ln2_b"], np.float64),
    )
    key = hashlib.sha256(b"".join(np.asarray(v).tobytes()
                                  for v in params.values())).hexdigest()
    if key not in _CACHE:
        consts, flags = _host_plan(**params)
        nc = _build_module(flags)
        _CACHE[key] = (_make_runner(nc), consts, flags)
    run, consts, flags = _CACHE[key]

    outs = run(_core_inputs(emb, pos, consts))
    out = np.asarray(outs["out"], dtype=np.float32)
    if flags["ln2_aff"]:
        g2, b2 = flags["ln2_vals"]
        out = out * g2.astype(np.float32) + b2.astype(np.float32)
    return np.ascontiguousarray(out.reshape(B, N, D))
